# revision 10
# baseline (speedup 1.0000x reference)
"""Trainium2 Bass kernel for nn_Encoder_57380763074770.

GRU-cell encoder over 64 independent "steps":
  xi  = concat(x[64,17], ip_emb[ip].reshape(64,8), port_emb[port].reshape(64,8))
  G   = xi @ W_ih.T + h0 @ W_hh.T + (b_ih + b_hh)       # [64, 384]
  r, z = sigmoid(G_r), sigmoid(G_z)
  n   = tanh(G_n + (r - 1) * hn),  hn = h0 @ W_hh_n.T + b_hh_n
  out = n + z * (h0 - n)                                # [64, 128]

Sharding: H=128 hidden columns split 8 ways -> each core owns 16 columns of
every gate (48 rows of W_ih/W_hh) and computes out[:, 16c:16c+16].

Layout decisions (driven by the HW profile -- fixed costs dominate at this
size: ~0.6us per dma_start issue, ~1.5-2us DMA completion latency, ~1.1us
per indirect DMA on the Q7, ~1.3us per activation-table load):
- Params ride in ONE packed [128, 451] f32 DMA on Sync; the DVE-gather
  blocks (iota, replicated 256-entry ip table, f32 ip indices) ride in a
  [64, 520] DMA issued on the Scalar-engine HWDGE in parallel; the port
  indices ride in a tiny [128, 1] DMA.
- The 512 ip_emb lookups (256-entry table) are computed on the VECTOR
  engine as one-hot compare + multiply + blocked reduce -- no indirect
  DMAs.  Only the port gather (70000x4 table) uses an indirect DMA:
  128 row lookups = 1 gather of 128 partitions.
- G is accumulated in PSUM with h-parts/biases/x first; only the last
  matmul waits on the DVE ip path.
- Activation tables pre-warmed with dummy ops so loads overlap the DMAs.
"""

import numpy as np

import concourse.bacc as bacc
import concourse.bass as bass
import concourse.mybir as mybir
import concourse.tile as tile
from concourse.bass_utils import run_bass_kernel_spmd

STEPS = 64
H = 128
N_CORES = 8
HS = H // N_CORES       # hidden cols per core = 16
G3 = 3 * HS             # gate rows per core = 48

F32 = mybir.dt.float32
I32 = mybir.dt.int32

# packed params column layout ("pk", [128, F_PK])
C_WIH = 0               # [0:48, 0:33]    W_ih slice (x | ip | port feature order)
C_WHH = 33              # [0:48, 33:161]  W_hh slice
C_ID = 161              # [0:128, 161:289] identity
C_X = 289               # [0:64, 289:306] x
C_H = 306               # [0:128, 306]    h0 column
C_B = 307               # [0, 307:355]    b_ih + b_hh slice
C_BN = 355              # [0, 355:371]    b_hh n-gate slice
C_H0 = 371              # [0, 371:387]    h0 slice for this core
C_ONE = 387             # [0, 387:451]    ones row
F_PK = 451

# DVE-gather block layout ("dv", [128, F_DV])
D_IOT = 0               # [:, 0:256]   iota row 0..255 per partition
D_EMB = 256             # [:, 256:512] ip_emb values replicated per row
D_IPF = 512             # [:, 512:516] ip[s, 4j+g] as f32 at partition j*64+s
F_DV = 516

V = 256                 # ip table size


def build_nc():
    nc = bacc.Bacc(None)

    idx_d = nc.declare_dram_parameter("idx", [H, 1], I32, isOutput=False)
    dv_d = nc.declare_dram_parameter("dv", [H, F_DV], F32, isOutput=False)
    pk_d = nc.declare_dram_parameter("pk", [H, F_PK], F32, isOutput=False)
    pte_d = nc.declare_dram_parameter("port_emb", [70000, 4], F32, isOutput=False)
    out_d = nc.declare_dram_parameter("out", [STEPS, HS], F32, isOutput=True)

    with tile.TileContext(nc) as tc:
        with (
            tc.tile_pool(name="sb", bufs=1) as sb,
            tc.tile_pool(name="ps", bufs=1, space="PSUM") as ps,
        ):
            idx = sb.tile([H, 1], I32)
            dv = sb.tile([H, F_DV], F32)
            pk = sb.tile([H, F_PK], F32)
            st = sb.tile([H, 4], F32)
            warm = sb.tile([1, 2], F32)
            eq = sb.tile([H, 4 * V], F32)
            pr = sb.tile([H, 4 * V], F32)
            ipf = sb.tile([H, 4], F32)
            whhT = sb.tile([H, G3], F32)
            w_x = sb.tile([17, G3], F32)
            w_ip0 = sb.tile([4, G3], F32)
            w_ip1 = sb.tile([4, G3], F32)
            w_pt0 = sb.tile([4, G3], F32)
            w_pt1 = sb.tile([4, G3], F32)
            xT = sb.tile([17, STEPS], F32)
            ipT = sb.tile([4, H], F32)
            ptT = sb.tile([4, H], F32)
            rz = sb.tile([STEPS, 2 * HS], F32)
            t2 = sb.tile([STEPS, HS], F32)
            u = sb.tile([STEPS, HS], F32)
            n = sb.tile([STEPS, HS], F32)
            zz = sb.tile([STEPS, HS], F32)
            zh = sb.tile([STEPS, HS], F32)
            m = sb.tile([STEPS, HS], F32)
            o = sb.tile([STEPS, HS], F32)

            # DMAs: idx on Sync, dv on Scalar HWDGE (parallel), pk on Sync
            nc.sync.dma_start(out=idx[:], in_=idx_d[:, :])
            nc.scalar.dma_start(out=dv[:], in_=dv_d[:, :])
            nc.sync.dma_start(out=pk[:], in_=pk_d[:, :])

            # the single port gather (gpsimd SWDGE, one row index per partition)
            nc.gpsimd.indirect_dma_start(
                out=st[:],
                out_offset=None,
                in_=pte_d[:, :],
                in_offset=bass.IndirectOffsetOnAxis(ap=idx[:, :], axis=0),
            )

            # pre-warm both activation tables
            nc.scalar.activation(warm[:, 0:1], dv[0:1, 0:1],
                                 mybir.ActivationFunctionType.Tanh)
            nc.scalar.activation(warm[:, 1:2], dv[0:1, 0:1],
                                 mybir.ActivationFunctionType.Sigmoid)

            # ip embedding on DVE (128 partitions: p = j*64+s holds k = 4j+g):
            # one-hot compare, then per-k fused multiply+reduce
            A = mybir.AluOpType
            ipb = dv[:, D_IPF : D_IPF + 4].unsqueeze(2).broadcast_to([H, 4, V])
            iob = dv[:, D_IOT : D_IOT + V].unsqueeze(1).broadcast_to([H, 4, V])
            emb = dv[:, D_EMB : D_EMB + V]
            eq3 = eq[:, :].rearrange("p (k v) -> p k v", v=V)
            pr3 = pr[:, :].rearrange("p (k v) -> p k v", v=V)
            emb3 = emb.unsqueeze(1).broadcast_to([H, 4, V])
            nc.vector.tensor_tensor(out=eq3, in0=ipb, in1=iob, op=A.is_equal)
            nc.vector.tensor_tensor(out=pr3, in0=eq3, in1=emb3, op=A.mult)
            nc.vector.tensor_reduce(out=ipf[:, :].unsqueeze(2), in_=pr3,
                                    axis=mybir.AxisListType.X, op=A.add)

            ident = pk[:, C_ID : C_ID + H]
            id48 = ident[:G3, :G3]

            # weight transposes (PE), copies on ACT
            p_whhT = ps.tile([H, G3], F32, space="PSUM", tag="wt", bufs=2)
            nc.tensor.transpose(out=p_whhT[:], in_=pk[0:G3, C_WHH : C_WHH + H],
                                identity=id48)
            nc.scalar.copy(out=whhT[:], in_=p_whhT[:])

            p_wx = ps.tile([17, G3], F32, space="PSUM", tag="wt", bufs=2)
            nc.tensor.transpose(out=p_wx[:], in_=pk[0:G3, C_WIH : C_WIH + 17],
                                identity=id48)
            nc.scalar.copy(out=w_x[:], in_=p_wx[:])

            p_wip0 = ps.tile([4, G3], F32, space="PSUM", tag="wt", bufs=2)
            nc.tensor.transpose(out=p_wip0[:], in_=pk[0:G3, C_WIH + 17 : C_WIH + 21],
                                identity=id48)
            nc.scalar.copy(out=w_ip0[:], in_=p_wip0[:])

            p_wip1 = ps.tile([4, G3], F32, space="PSUM", tag="wt", bufs=2)
            nc.tensor.transpose(out=p_wip1[:], in_=pk[0:G3, C_WIH + 21 : C_WIH + 25],
                                identity=id48)
            nc.scalar.copy(out=w_ip1[:], in_=p_wip1[:])

            p_wp0 = ps.tile([4, G3], F32, space="PSUM", tag="wt", bufs=2)
            nc.tensor.transpose(out=p_wp0[:], in_=pk[0:G3, C_WIH + 25 : C_WIH + 29],
                                identity=id48)
            nc.scalar.copy(out=w_pt0[:], in_=p_wp0[:])

            p_wp1 = ps.tile([4, G3], F32, space="PSUM", tag="wt", bufs=2)
            nc.tensor.transpose(out=p_wp1[:], in_=pk[0:G3, C_WIH + 29 : C_WIH + 33],
                                identity=id48)
            nc.scalar.copy(out=w_pt1[:], in_=p_wp1[:])

            # x transpose, early
            p_xT = ps.tile([17, STEPS], F32, space="PSUM", tag="wt", bufs=2)
            nc.tensor.transpose(out=p_xT[:], in_=pk[0:STEPS, C_X : C_X + 17],
                                identity=ident[:STEPS, :STEPS])
            nc.scalar.copy(out=xT[:], in_=p_xT[:])

            hcol_b = pk[:, C_H : C_H + 1].to_broadcast([H, STEPS])
            ones = pk[0:1, C_ONE : C_ONE + STEPS]

            # h-dependent matmuls
            HN = ps.tile([STEPS, HS], F32, space="PSUM")
            nc.tensor.matmul(out=HN[:], lhsT=hcol_b, rhs=whhT[:, 2 * HS : 3 * HS],
                             start=True, stop=False)
            nc.tensor.matmul(out=HN[:], lhsT=ones, rhs=pk[0:1, C_BN : C_BN + HS],
                             start=False, stop=True)

            H0B = ps.tile([STEPS, HS], F32, space="PSUM")
            nc.tensor.matmul(out=H0B[:], lhsT=ones, rhs=pk[0:1, C_H0 : C_H0 + HS],
                             start=True, stop=True)

            # port transpose (after the gather)
            p_ptT = ps.tile([4, H], F32, space="PSUM")
            nc.tensor.transpose(out=p_ptT[:], in_=st[:], identity=ident)
            nc.scalar.copy(out=ptT[:], in_=p_ptT[:])

            # G accumulation: everything not ip-dependent first
            G = ps.tile([STEPS, G3], F32, space="PSUM")
            nc.tensor.matmul(out=G[:], lhsT=hcol_b, rhs=whhT[:], start=True, stop=False)
            nc.tensor.matmul(out=G[:], lhsT=ones, rhs=pk[0:1, C_B : C_B + G3],
                             start=False, stop=False)
            nc.tensor.matmul(out=G[:], lhsT=xT[:], rhs=w_x[:], start=False, stop=False)
            nc.tensor.matmul(out=G[:], lhsT=ptT[:, 0:STEPS], rhs=w_pt0[:],
                             start=False, stop=False)
            nc.tensor.matmul(out=G[:], lhsT=ptT[:, STEPS : 2 * STEPS], rhs=w_pt1[:],
                             start=False, stop=False)

            # ip transpose + final G matmuls
            p_ipT = ps.tile([4, H], F32, space="PSUM")
            nc.tensor.transpose(out=p_ipT[:], in_=ipf[:, :], identity=ident)
            nc.scalar.copy(out=ipT[:], in_=p_ipT[:])
            nc.tensor.matmul(out=G[:], lhsT=ipT[:, 0:STEPS], rhs=w_ip0[:],
                             start=False, stop=False)
            nc.tensor.matmul(out=G[:], lhsT=ipT[:, STEPS : 2 * STEPS], rhs=w_ip1[:],
                             start=False, stop=True)

            # gates
            nc.scalar.activation(rz[:], G[:, 0 : 2 * HS],
                                 mybir.ActivationFunctionType.Sigmoid)
            r = rz[:, 0:HS]
            z = rz[:, HS : 2 * HS]
            nc.vector.scalar_tensor_tensor(
                out=t2[:], in0=r, scalar=1.0, in1=HN[:], op0=A.subtract, op1=A.mult)
            nc.vector.tensor_add(out=u[:], in0=G[:, 2 * HS : 3 * HS], in1=t2[:])
            # overlap with tanh: zz = z-1, zh = z*h0
            nc.vector.tensor_scalar_add(out=zz[:], in0=z, scalar1=-1.0)
            nc.vector.tensor_mul(out=zh[:], in0=z, in1=H0B[:])
            nc.scalar.activation(n[:], u[:], mybir.ActivationFunctionType.Tanh)
            # o = z*h0 - n*(z-1) = n + z*(h0-n)
            nc.vector.tensor_mul(out=m[:], in0=n[:], in1=zz[:])
            nc.vector.tensor_sub(out=o[:], in0=zh[:], in1=m[:])

            nc.sync.dma_start(out=out_d[:, :], in_=o[:])

    nc.finalize()
    return nc


def make_in_maps(inputs):
    x = np.asarray(inputs["x"], dtype=np.float32)
    ipi = np.asarray(inputs["ip"], dtype=np.int32)
    pti = np.asarray(inputs["port"], dtype=np.int32)
    hid = np.asarray(inputs["hidden"], dtype=np.float32).reshape(H)
    ip_emb = np.ascontiguousarray(np.asarray(inputs["ip_emb"], dtype=np.float32))
    port_emb = np.ascontiguousarray(np.asarray(inputs["port_emb"], dtype=np.float32))
    W_ih = np.asarray(inputs["W_ih"], dtype=np.float32)
    W_hh = np.asarray(inputs["W_hh"], dtype=np.float32)
    b = np.asarray(inputs["b_ih"], dtype=np.float32) + np.asarray(
        inputs["b_hh"], dtype=np.float32
    )
    b_hh = np.asarray(inputs["b_hh"], dtype=np.float32)

    idx = np.ascontiguousarray(pti.T.reshape(H, 1))  # port[s,k] at partition k*64+s

    dv = np.zeros((H, F_DV), dtype=np.float32)
    dv[:, D_IOT : D_IOT + V] = np.arange(V, dtype=np.float32)
    dv[:, D_EMB : D_EMB + V] = ip_emb[:, 0]
    dv[:, D_IPF : D_IPF + 4] = (
        ipi.reshape(STEPS, 2, 4).transpose(1, 0, 2).reshape(H, 4).astype(np.float32)
    )

    in_maps = []
    for c in range(N_CORES):
        sl = np.arange(c * HS, (c + 1) * HS)
        rows = np.concatenate([sl, H + sl, 2 * H + sl])
        pk = np.zeros((H, F_PK), dtype=np.float32)
        pk[0:G3, C_WIH : C_WIH + 33] = W_ih[rows]
        pk[0:G3, C_WHH : C_WHH + H] = W_hh[rows]
        pk[:, C_ID : C_ID + H] = np.eye(H, dtype=np.float32)
        pk[0:STEPS, C_X : C_X + 17] = x
        pk[:, C_H] = hid
        pk[0, C_B : C_B + G3] = b[rows]
        pk[0, C_BN : C_BN + HS] = b_hh[2 * H + sl]
        pk[0, C_H0 : C_H0 + HS] = hid[sl]
        pk[0, C_ONE : C_ONE + STEPS] = 1.0
        in_maps.append(
            {"idx": idx, "dv": dv, "pk": pk, "port_emb": port_emb}
        )
    return in_maps


_NC = None


def run(inputs, trace=False):
    global _NC
    if _NC is None:
        _NC = build_nc()
    res = run_bass_kernel_spmd(_NC, make_in_maps(inputs), list(range(N_CORES)), trace=trace)
    outputs = np.concatenate([res.results[c]["out"] for c in range(N_CORES)], axis=1)
    new_hidden = np.ascontiguousarray(outputs[STEPS - 1].reshape(1, 1, H))
    return (outputs, new_hidden), res


def kernel(**inputs):
    (outputs, new_hidden), _ = run(inputs)
    return outputs, new_hidden


# revision 11
# speedup vs baseline: 1.1290x; 1.1290x over previous
"""Trainium2 Bass kernel for nn_Encoder_57380763074770.

GRU-cell encoder over 64 independent "steps":
  xi  = concat(x[64,17], ip_emb[ip].reshape(64,8), port_emb[port].reshape(64,8))
  G   = xi @ W_ih.T + h0 @ W_hh.T + (b_ih + b_hh)       # [64, 384]
  r, z = sigmoid(G_r), sigmoid(G_z)
  n   = tanh(G_n + (r - 1) * hn),  hn = h0 @ W_hh_n.T + b_hh_n
  out = n + z * (h0 - n)                                # [64, 128]

Sharding: H=128 hidden columns split 8 ways -> each core owns 16 columns of
every gate (48 rows of W_ih/W_hh) and computes out[:, 16c:16c+16].

Layout decisions (driven by the HW profile -- fixed costs dominate at this
size: ~0.6us per dma_start issue, ~1.5-2us DMA completion latency, ~1.1us
per indirect DMA on the Q7, ~1.3us per activation-table load):
- Weights/x ride HOST-TRANSPOSED (contraction-major) inside ONE packed
  [128, 625] f32 DMA, so the PE does no weight transposes at all.
- Port indices and ip indices ride in one tiny [128, 5] i32 DMA that lands
  first; the replicated 256-entry ip table rides on the Scalar-engine HWDGE
  in parallel.
- The 512 ip_emb lookups are computed on the VECTOR engine (128 partitions,
  partition j*64+s holds columns k=4j+g): int32 iota (generated on GpSimd)
  + one-hot is_equal + multiply + blocked 3D reduce.
- The port gather (70000x4 table) is ONE indirect DMA of 128 row lookups.
- G is accumulated in PSUM h-parts/bias/x first; only the final two
  matmuls wait on the DVE ip path.
- Activation tables pre-warmed against a memset scratch so both loads
  overlap the input DMAs.
"""

import numpy as np

import concourse.bacc as bacc
import concourse.bass as bass
import concourse.mybir as mybir
import concourse.tile as tile
from concourse.bass_utils import run_bass_kernel_spmd

STEPS = 64
H = 128
N_CORES = 8
HS = H // N_CORES       # hidden cols per core = 16
G3 = 3 * HS             # gate rows per core = 48

F32 = mybir.dt.float32
I32 = mybir.dt.int32

# packed params column layout ("pk", [128, F_PK]); all W blocks contraction-major
C_WHT = 0               # [0:128, 0:48]    W_hh slice, transposed
C_WX = 48               # [0:17, 48:96]    W_ih x-features, transposed
C_WIP0 = 96             # [0:4, 96:144]    W_ih ip k=0..3, transposed
C_WIP1 = 144            # [0:4, 144:192]   W_ih ip k=4..7, transposed
C_WPT0 = 192            # [0:4, 192:240]   W_ih port k=0, transposed
C_WPT1 = 240            # [0:4, 240:288]   W_ih port k=1, transposed
C_XT = 288              # [0:17, 288:352]  x transposed
C_ID = 352              # [0:128, 352:480] identity
C_H = 480               # [0:128, 480]     h0 column
C_B = 481               # [0, 481:529]     b_ih + b_hh slice
C_BN = 529              # [0, 529:545]     b_hh n-gate slice
C_H0 = 545              # [0, 545:561]     h0 slice for this core
C_ONE = 561             # [0, 561:625]     ones row
F_PK = 625

V = 256                 # ip table size


def build_nc():
    nc = bacc.Bacc(None)

    idx_d = nc.declare_dram_parameter("idx", [H, 5], I32, isOutput=False)
    dv_d = nc.declare_dram_parameter("dv", [H, V], F32, isOutput=False)
    pk_d = nc.declare_dram_parameter("pk", [H, F_PK], F32, isOutput=False)
    pte_d = nc.declare_dram_parameter("port_emb", [70000, 4], F32, isOutput=False)
    out_d = nc.declare_dram_parameter("out", [STEPS, HS], F32, isOutput=True)

    with tile.TileContext(nc) as tc:
        with (
            tc.tile_pool(name="sb", bufs=1) as sb,
            tc.tile_pool(name="ps", bufs=1, space="PSUM") as ps,
        ):
            idx = sb.tile([H, 5], I32)
            dv = sb.tile([H, V], F32)
            pk = sb.tile([H, F_PK], F32)
            st = sb.tile([H, 4], F32)
            iot = sb.tile([H, V], I32)
            wsrc = sb.tile([1, 1], F32)
            warm = sb.tile([1, 2], F32)
            eq = sb.tile([H, 4 * V], F32)
            pr = sb.tile([H, 4 * V], F32)
            ipf = sb.tile([H, 4], F32)
            ipT = sb.tile([4, H], F32)
            ptT = sb.tile([4, H], F32)
            rz = sb.tile([STEPS, 2 * HS], F32)
            t2 = sb.tile([STEPS, HS], F32)
            u = sb.tile([STEPS, HS], F32)
            n = sb.tile([STEPS, HS], F32)
            zz = sb.tile([STEPS, HS], F32)
            zh = sb.tile([STEPS, HS], F32)
            m = sb.tile([STEPS, HS], F32)
            o = sb.tile([STEPS, HS], F32)

            # DMAs: idx + pk on Sync, dv (ip table) on Scalar HWDGE in parallel
            nc.sync.dma_start(out=idx[:], in_=idx_d[:, :])
            nc.scalar.dma_start(out=dv[:], in_=dv_d[:, :])
            nc.sync.dma_start(out=pk[:], in_=pk_d[:, :])

            # gpsimd: scratch memset, iota, then the single port gather
            nc.gpsimd.memset(wsrc[:], 0.25)
            nc.gpsimd.iota(iot[:], pattern=[[1, V]], base=0, channel_multiplier=0)
            nc.gpsimd.indirect_dma_start(
                out=st[:],
                out_offset=None,
                in_=pte_d[:, :],
                in_offset=bass.IndirectOffsetOnAxis(ap=idx[:, 0:1], axis=0),
            )

            # pre-warm both activation tables (reads only the memset scratch)
            nc.scalar.activation(warm[:, 0:1], wsrc[:],
                                 mybir.ActivationFunctionType.Tanh)
            nc.scalar.activation(warm[:, 1:2], wsrc[:],
                                 mybir.ActivationFunctionType.Sigmoid)

            # ip embedding on DVE (partition j*64+s holds k = 4j+g):
            # int one-hot compare, multiply by table, blocked reduce
            A = mybir.AluOpType
            ipb = idx[:, 1:5].unsqueeze(2).broadcast_to([H, 4, V])
            iob = iot[:, :].unsqueeze(1).broadcast_to([H, 4, V])
            emb3 = dv[:, :].unsqueeze(1).broadcast_to([H, 4, V])
            eq3 = eq[:, :].rearrange("p (k v) -> p k v", v=V)
            pr3 = pr[:, :].rearrange("p (k v) -> p k v", v=V)
            nc.vector.tensor_tensor(out=eq3, in0=ipb, in1=iob, op=A.is_equal)
            nc.vector.tensor_tensor(out=pr3, in0=eq3, in1=emb3, op=A.mult)
            nc.vector.tensor_reduce(out=ipf[:, :].unsqueeze(2), in_=pr3,
                                    axis=mybir.AxisListType.X, op=A.add)

            ident = pk[:, C_ID : C_ID + H]
            hcol_b = pk[:, C_H : C_H + 1].to_broadcast([H, STEPS])
            ones = pk[0:1, C_ONE : C_ONE + STEPS]

            # h-dependent matmuls (all inputs direct from pk)
            HN = ps.tile([STEPS, HS], F32, space="PSUM")
            nc.tensor.matmul(out=HN[:], lhsT=hcol_b,
                             rhs=pk[0:H, C_WHT + 2 * HS : C_WHT + 3 * HS],
                             start=True, stop=False)
            nc.tensor.matmul(out=HN[:], lhsT=ones, rhs=pk[0:1, C_BN : C_BN + HS],
                             start=False, stop=True)

            H0B = ps.tile([STEPS, HS], F32, space="PSUM")
            nc.tensor.matmul(out=H0B[:], lhsT=ones, rhs=pk[0:1, C_H0 : C_H0 + HS],
                             start=True, stop=True)

            # port transpose (after the gather)
            p_ptT = ps.tile([4, H], F32, space="PSUM")
            nc.tensor.transpose(out=p_ptT[:], in_=st[:], identity=ident)
            nc.scalar.copy(out=ptT[:], in_=p_ptT[:])

            # G accumulation: everything not ip-dependent first
            G = ps.tile([STEPS, G3], F32, space="PSUM")
            nc.tensor.matmul(out=G[:], lhsT=hcol_b, rhs=pk[0:H, C_WHT : C_WHT + G3],
                             start=True, stop=False)
            nc.tensor.matmul(out=G[:], lhsT=ones, rhs=pk[0:1, C_B : C_B + G3],
                             start=False, stop=False)
            nc.tensor.matmul(out=G[:], lhsT=pk[0:17, C_XT : C_XT + STEPS],
                             rhs=pk[0:17, C_WX : C_WX + G3], start=False, stop=False)
            nc.tensor.matmul(out=G[:], lhsT=ptT[:, 0:STEPS],
                             rhs=pk[0:4, C_WPT0 : C_WPT0 + G3], start=False, stop=False)
            nc.tensor.matmul(out=G[:], lhsT=ptT[:, STEPS : 2 * STEPS],
                             rhs=pk[0:4, C_WPT1 : C_WPT1 + G3], start=False, stop=False)

            # ip transpose + final G matmuls
            p_ipT = ps.tile([4, H], F32, space="PSUM")
            nc.tensor.transpose(out=p_ipT[:], in_=ipf[:, :], identity=ident)
            nc.scalar.copy(out=ipT[:], in_=p_ipT[:])
            nc.tensor.matmul(out=G[:], lhsT=ipT[:, 0:STEPS],
                             rhs=pk[0:4, C_WIP0 : C_WIP0 + G3], start=False, stop=False)
            nc.tensor.matmul(out=G[:], lhsT=ipT[:, STEPS : 2 * STEPS],
                             rhs=pk[0:4, C_WIP1 : C_WIP1 + G3], start=False, stop=True)

            # gates
            nc.scalar.activation(rz[:], G[:, 0 : 2 * HS],
                                 mybir.ActivationFunctionType.Sigmoid)
            r = rz[:, 0:HS]
            z = rz[:, HS : 2 * HS]
            nc.vector.scalar_tensor_tensor(
                out=t2[:], in0=r, scalar=1.0, in1=HN[:], op0=A.subtract, op1=A.mult)
            nc.vector.tensor_add(out=u[:], in0=G[:, 2 * HS : 3 * HS], in1=t2[:])
            nc.vector.tensor_scalar_add(out=zz[:], in0=z, scalar1=-1.0)
            nc.vector.tensor_mul(out=zh[:], in0=z, in1=H0B[:])
            nc.scalar.activation(n[:], u[:], mybir.ActivationFunctionType.Tanh)
            # o = z*h0 - n*(z-1) = n + z*(h0-n)
            nc.vector.tensor_mul(out=m[:], in0=n[:], in1=zz[:])
            nc.vector.tensor_sub(out=o[:], in0=zh[:], in1=m[:])

            nc.sync.dma_start(out=out_d[:, :], in_=o[:])

    nc.finalize()
    return nc


def make_in_maps(inputs):
    x = np.asarray(inputs["x"], dtype=np.float32)
    ipi = np.asarray(inputs["ip"], dtype=np.int32)
    pti = np.asarray(inputs["port"], dtype=np.int32)
    hid = np.asarray(inputs["hidden"], dtype=np.float32).reshape(H)
    ip_emb = np.ascontiguousarray(np.asarray(inputs["ip_emb"], dtype=np.float32))
    port_emb = np.ascontiguousarray(np.asarray(inputs["port_emb"], dtype=np.float32))
    W_ih = np.asarray(inputs["W_ih"], dtype=np.float32)
    W_hh = np.asarray(inputs["W_hh"], dtype=np.float32)
    b = np.asarray(inputs["b_ih"], dtype=np.float32) + np.asarray(
        inputs["b_hh"], dtype=np.float32
    )
    b_hh = np.asarray(inputs["b_hh"], dtype=np.float32)

    idx = np.zeros((H, 5), dtype=np.int32)
    idx[:, 0] = pti.T.reshape(H)                 # port[s,k] at partition k*64+s
    idx[:, 1:5] = ipi.reshape(STEPS, 2, 4).transpose(1, 0, 2).reshape(H, 4)

    dv = np.broadcast_to(ip_emb[:, 0], (H, V)).copy()

    in_maps = []
    for c in range(N_CORES):
        sl = np.arange(c * HS, (c + 1) * HS)
        rows = np.concatenate([sl, H + sl, 2 * H + sl])
        pk = np.zeros((H, F_PK), dtype=np.float32)
        pk[0:H, C_WHT : C_WHT + G3] = W_hh[rows].T
        pk[0:17, C_WX : C_WX + G3] = W_ih[rows, 0:17].T
        pk[0:4, C_WIP0 : C_WIP0 + G3] = W_ih[rows, 17:21].T
        pk[0:4, C_WIP1 : C_WIP1 + G3] = W_ih[rows, 21:25].T
        pk[0:4, C_WPT0 : C_WPT0 + G3] = W_ih[rows, 25:29].T
        pk[0:4, C_WPT1 : C_WPT1 + G3] = W_ih[rows, 29:33].T
        pk[0:17, C_XT : C_XT + STEPS] = x.T
        pk[:, C_ID : C_ID + H] = np.eye(H, dtype=np.float32)
        pk[:, C_H] = hid
        pk[0, C_B : C_B + G3] = b[rows]
        pk[0, C_BN : C_BN + HS] = b_hh[2 * H + sl]
        pk[0, C_H0 : C_H0 + HS] = hid[sl]
        pk[0, C_ONE : C_ONE + STEPS] = 1.0
        in_maps.append(
            {"idx": idx, "dv": dv, "pk": pk, "port_emb": port_emb}
        )
    return in_maps


_NC = None


def run(inputs, trace=False):
    global _NC
    if _NC is None:
        _NC = build_nc()
    res = run_bass_kernel_spmd(_NC, make_in_maps(inputs), list(range(N_CORES)), trace=trace)
    outputs = np.concatenate([res.results[c]["out"] for c in range(N_CORES)], axis=1)
    new_hidden = np.ascontiguousarray(outputs[STEPS - 1].reshape(1, 1, H))
    return (outputs, new_hidden), res


def kernel(**inputs):
    (outputs, new_hidden), _ = run(inputs)
    return outputs, new_hidden


# revision 12
# speedup vs baseline: 1.1743x; 1.0401x over previous
"""Trainium2 Bass kernel for nn_Encoder_57380763074770.

GRU-cell encoder over 64 independent "steps":
  xi  = concat(x[64,17], ip_emb[ip].reshape(64,8), port_emb[port].reshape(64,8))
  G   = xi @ W_ih.T + h0 @ W_hh.T + (b_ih + b_hh)       # [64, 384]
  r, z = sigmoid(G_r), sigmoid(G_z)
  n   = tanh(G_n + (r - 1) * hn),  hn = h0 @ W_hh_n.T + b_hh_n
  out = n + z * (h0 - n)                                # [64, 128]

Sharding: H=128 hidden columns split 8 ways -> each core owns 16 columns of
every gate (48 rows of W_ih/W_hh) and computes out[:, 16c:16c+16].

Layout decisions (driven by the HW profile -- fixed costs dominate at this
size: ~0.6us per dma_start issue, ~1.5-2us DMA completion latency, ~1.1us
per indirect DMA on the Q7, ~1.3us per activation-table load):
- Weights/x ride HOST-TRANSPOSED (contraction-major) inside ONE packed
  [128, 625] f32 DMA, so the PE does no weight transposes at all.
- Port indices and ip indices ride in one tiny [128, 5] i32 DMA that lands
  first; the replicated 256-entry ip table rides on the Scalar-engine HWDGE
  in parallel.
- The 512 ip_emb lookups are computed on the VECTOR engine (128 partitions,
  partition j*64+s holds columns k=4j+g): int32 iota (generated on GpSimd)
  + one-hot is_equal + multiply + blocked 3D reduce.
- The port gather (70000x4 table) is ONE indirect DMA of 128 row lookups.
- G is accumulated in PSUM h-parts/bias/x first; only the final two
  matmuls wait on the DVE ip path.
- Activation tables pre-warmed against a memset scratch so both loads
  overlap the input DMAs.
"""

import numpy as np

import concourse.bacc as bacc
import concourse.bass as bass
import concourse.mybir as mybir
import concourse.tile as tile
from concourse.bass_utils import run_bass_kernel_spmd

STEPS = 64
H = 128
N_CORES = 8
HS = H // N_CORES       # hidden cols per core = 16
G3 = 3 * HS             # gate rows per core = 48

F32 = mybir.dt.float32
I32 = mybir.dt.int32

# packed params column layout ("pk", [128, F_PK]); all W blocks contraction-major
C_WHT = 0               # [0:128, 0:48]    W_hh slice, transposed
C_WX = 48               # [0:17, 48:96]    W_ih x-features, transposed
C_WIP0 = 96             # [0:4, 96:144]    W_ih ip k=0..3, transposed
C_WIP1 = 144            # [0:4, 144:192]   W_ih ip k=4..7, transposed
C_WPT0 = 192            # [0:4, 192:240]   W_ih port k=0, transposed
C_WPT1 = 240            # [0:4, 240:288]   W_ih port k=1, transposed
C_XT = 288              # [0:17, 288:352]  x transposed
C_ID = 352              # [0:128, 352:480] identity
C_H = 480               # [0:128, 480]     h0 column
C_B = 481               # [0, 481:529]     b_ih + b_hh slice
C_BN = 529              # [0, 529:545]     b_hh n-gate slice
C_H0 = 545              # [0, 545:561]     h0 slice for this core
C_ONE = 561             # [0, 561:625]     ones row
F_PK = 625

V = 256                 # ip table size


def build_nc():
    nc = bacc.Bacc(None)

    idx_d = nc.declare_dram_parameter("idx", [H, 5], I32, isOutput=False)
    dv_d = nc.declare_dram_parameter("dv", [H, V], F32, isOutput=False)
    pk_d = nc.declare_dram_parameter("pk", [H, F_PK], F32, isOutput=False)
    pte_d = nc.declare_dram_parameter("port_emb", [70000, 4], F32, isOutput=False)
    out_d = nc.declare_dram_parameter("out", [STEPS, HS], F32, isOutput=True)

    with tile.TileContext(nc) as tc:
        with (
            tc.tile_pool(name="sb", bufs=1) as sb,
            tc.tile_pool(name="ps", bufs=1, space="PSUM") as ps,
        ):
            idx = sb.tile([H, 5], I32)
            dv = sb.tile([H, V], F32)
            pk = sb.tile([H, F_PK], F32)
            st = sb.tile([H, 4], F32)
            iot = sb.tile([H, V], I32)
            wsrc = sb.tile([1, 1], F32)
            warm = sb.tile([1, 2], F32)
            eq = sb.tile([H, 4 * V], F32)
            pr = sb.tile([H, 4 * V], F32)
            ipf = sb.tile([H, 4], F32)
            ipT = sb.tile([4, H], F32)
            ptT = sb.tile([4, H], F32)
            rz = sb.tile([STEPS, 2 * HS], F32)
            t2 = sb.tile([STEPS, HS], F32)
            u = sb.tile([STEPS, HS], F32)
            n = sb.tile([STEPS, HS], F32)
            zz = sb.tile([STEPS, HS], F32)
            zh = sb.tile([STEPS, HS], F32)
            m = sb.tile([STEPS, HS], F32)
            o = sb.tile([STEPS, HS], F32)

            # DMAs: idx + pk on Sync, dv (ip table) on Scalar HWDGE in parallel
            nc.sync.dma_start(out=idx[:], in_=idx_d[:, :])
            nc.scalar.dma_start(out=dv[:], in_=dv_d[:, :])
            nc.sync.dma_start(out=pk[:], in_=pk_d[:, :])

            # gpsimd: scratch memset, iota, then the single port gather
            nc.gpsimd.memset(wsrc[:], 0.25)
            nc.gpsimd.iota(iot[:], pattern=[[1, V]], base=0, channel_multiplier=0)
            nc.gpsimd.indirect_dma_start(
                out=st[:],
                out_offset=None,
                in_=pte_d[:, :],
                in_offset=bass.IndirectOffsetOnAxis(ap=idx[:, 0:1], axis=0),
            )

            # pre-warm both activation tables (reads only the memset scratch)
            nc.scalar.activation(warm[:, 0:1], wsrc[:],
                                 mybir.ActivationFunctionType.Tanh)
            nc.scalar.activation(warm[:, 1:2], wsrc[:],
                                 mybir.ActivationFunctionType.Sigmoid)

            # ip embedding on DVE (partition j*64+s holds k = 4j+g):
            # int one-hot compare, multiply by table, blocked reduce
            A = mybir.AluOpType
            ipb = idx[:, 1:5].unsqueeze(2).broadcast_to([H, 4, V])
            iob = iot[:, :].unsqueeze(1).broadcast_to([H, 4, V])
            emb3 = dv[:, :].unsqueeze(1).broadcast_to([H, 4, V])
            eq3 = eq[:, :].rearrange("p (k v) -> p k v", v=V)
            pr3 = pr[:, :].rearrange("p (k v) -> p k v", v=V)
            nc.vector.tensor_tensor(out=eq3, in0=ipb, in1=iob, op=A.is_equal)
            nc.vector.tensor_tensor(out=pr3, in0=eq3, in1=emb3, op=A.mult)
            nc.vector.tensor_reduce(out=ipf[:, :].unsqueeze(2), in_=pr3,
                                    axis=mybir.AxisListType.X, op=A.add)

            ident = pk[:, C_ID : C_ID + H]
            hcol_b = pk[:, C_H : C_H + 1].to_broadcast([H, STEPS])
            ones = pk[0:1, C_ONE : C_ONE + STEPS]

            # h-dependent matmuls (all inputs direct from pk)
            HN = ps.tile([STEPS, HS], F32, space="PSUM")
            nc.tensor.matmul(out=HN[:], lhsT=hcol_b,
                             rhs=pk[0:H, C_WHT + 2 * HS : C_WHT + 3 * HS],
                             start=True, stop=False)
            nc.tensor.matmul(out=HN[:], lhsT=ones, rhs=pk[0:1, C_BN : C_BN + HS],
                             start=False, stop=True)

            H0B = ps.tile([STEPS, HS], F32, space="PSUM")
            nc.tensor.matmul(out=H0B[:], lhsT=ones, rhs=pk[0:1, C_H0 : C_H0 + HS],
                             start=True, stop=True)

            # G accumulation: everything not gather/ip-dependent first
            G = ps.tile([STEPS, G3], F32, space="PSUM")
            nc.tensor.matmul(out=G[:], lhsT=hcol_b, rhs=pk[0:H, C_WHT : C_WHT + G3],
                             start=True, stop=False)
            nc.tensor.matmul(out=G[:], lhsT=ones, rhs=pk[0:1, C_B : C_B + G3],
                             start=False, stop=False)
            nc.tensor.matmul(out=G[:], lhsT=pk[0:17, C_XT : C_XT + STEPS],
                             rhs=pk[0:17, C_WX : C_WX + G3], start=False, stop=False)

            # transposes of the gathered/computed embeddings, then their matmuls
            p_ptT = ps.tile([4, H], F32, space="PSUM")
            nc.tensor.transpose(out=p_ptT[:], in_=st[:], identity=ident)
            nc.vector.tensor_copy(out=ptT[:], in_=p_ptT[:])
            p_ipT = ps.tile([4, H], F32, space="PSUM")
            nc.tensor.transpose(out=p_ipT[:], in_=ipf[:, :], identity=ident)
            nc.vector.tensor_copy(out=ipT[:], in_=p_ipT[:])
            nc.tensor.matmul(out=G[:], lhsT=ptT[:, 0:STEPS],
                             rhs=pk[0:4, C_WPT0 : C_WPT0 + G3], start=False, stop=False)
            nc.tensor.matmul(out=G[:], lhsT=ptT[:, STEPS : 2 * STEPS],
                             rhs=pk[0:4, C_WPT1 : C_WPT1 + G3], start=False, stop=False)
            nc.tensor.matmul(out=G[:], lhsT=ipT[:, 0:STEPS],
                             rhs=pk[0:4, C_WIP0 : C_WIP0 + G3], start=False, stop=False)
            nc.tensor.matmul(out=G[:], lhsT=ipT[:, STEPS : 2 * STEPS],
                             rhs=pk[0:4, C_WIP1 : C_WIP1 + G3], start=False, stop=True)

            # gates
            nc.scalar.activation(rz[:], G[:, 0 : 2 * HS],
                                 mybir.ActivationFunctionType.Sigmoid)
            r = rz[:, 0:HS]
            z = rz[:, HS : 2 * HS]
            nc.vector.scalar_tensor_tensor(
                out=t2[:], in0=r, scalar=1.0, in1=HN[:], op0=A.subtract, op1=A.mult)
            nc.vector.tensor_add(out=u[:], in0=G[:, 2 * HS : 3 * HS], in1=t2[:])
            nc.vector.tensor_scalar_add(out=zz[:], in0=z, scalar1=-1.0)
            nc.vector.tensor_mul(out=zh[:], in0=z, in1=H0B[:])
            nc.scalar.activation(n[:], u[:], mybir.ActivationFunctionType.Tanh)
            # o = z*h0 - n*(z-1) = n + z*(h0-n)
            nc.vector.tensor_mul(out=m[:], in0=n[:], in1=zz[:])
            nc.vector.tensor_sub(out=o[:], in0=zh[:], in1=m[:])

            nc.sync.dma_start(out=out_d[:, :], in_=o[:])

    nc.finalize()
    return nc


def make_in_maps(inputs):
    x = np.asarray(inputs["x"], dtype=np.float32)
    ipi = np.asarray(inputs["ip"], dtype=np.int32)
    pti = np.asarray(inputs["port"], dtype=np.int32)
    hid = np.asarray(inputs["hidden"], dtype=np.float32).reshape(H)
    ip_emb = np.ascontiguousarray(np.asarray(inputs["ip_emb"], dtype=np.float32))
    port_emb = np.ascontiguousarray(np.asarray(inputs["port_emb"], dtype=np.float32))
    W_ih = np.asarray(inputs["W_ih"], dtype=np.float32)
    W_hh = np.asarray(inputs["W_hh"], dtype=np.float32)
    b = np.asarray(inputs["b_ih"], dtype=np.float32) + np.asarray(
        inputs["b_hh"], dtype=np.float32
    )
    b_hh = np.asarray(inputs["b_hh"], dtype=np.float32)

    idx = np.zeros((H, 5), dtype=np.int32)
    idx[:, 0] = pti.T.reshape(H)                 # port[s,k] at partition k*64+s
    idx[:, 1:5] = ipi.reshape(STEPS, 2, 4).transpose(1, 0, 2).reshape(H, 4)

    dv = np.broadcast_to(ip_emb[:, 0], (H, V)).copy()

    in_maps = []
    for c in range(N_CORES):
        sl = np.arange(c * HS, (c + 1) * HS)
        rows = np.concatenate([sl, H + sl, 2 * H + sl])
        pk = np.zeros((H, F_PK), dtype=np.float32)
        pk[0:H, C_WHT : C_WHT + G3] = W_hh[rows].T
        pk[0:17, C_WX : C_WX + G3] = W_ih[rows, 0:17].T
        pk[0:4, C_WIP0 : C_WIP0 + G3] = W_ih[rows, 17:21].T
        pk[0:4, C_WIP1 : C_WIP1 + G3] = W_ih[rows, 21:25].T
        pk[0:4, C_WPT0 : C_WPT0 + G3] = W_ih[rows, 25:29].T
        pk[0:4, C_WPT1 : C_WPT1 + G3] = W_ih[rows, 29:33].T
        pk[0:17, C_XT : C_XT + STEPS] = x.T
        pk[:, C_ID : C_ID + H] = np.eye(H, dtype=np.float32)
        pk[:, C_H] = hid
        pk[0, C_B : C_B + G3] = b[rows]
        pk[0, C_BN : C_BN + HS] = b_hh[2 * H + sl]
        pk[0, C_H0 : C_H0 + HS] = hid[sl]
        pk[0, C_ONE : C_ONE + STEPS] = 1.0
        in_maps.append(
            {"idx": idx, "dv": dv, "pk": pk, "port_emb": port_emb}
        )
    return in_maps


_NC = None


def run(inputs, trace=False):
    global _NC
    if _NC is None:
        _NC = build_nc()
    res = run_bass_kernel_spmd(_NC, make_in_maps(inputs), list(range(N_CORES)), trace=trace)
    outputs = np.concatenate([res.results[c]["out"] for c in range(N_CORES)], axis=1)
    new_hidden = np.ascontiguousarray(outputs[STEPS - 1].reshape(1, 1, H))
    return (outputs, new_hidden), res


def kernel(**inputs):
    (outputs, new_hidden), _ = run(inputs)
    return outputs, new_hidden
